# revision 8
# baseline (speedup 1.0000x reference)
"""DECConsLoss Trainium2 kernel: 8-core data-parallel over groups.

Reference computation (per group g of G=32, M=2048 tokens, C=512):
  ft_n, fc_n = l2norm(ft), l2norm(fc)          [M, C]
  grp[m]     = argmax_s grp_masks[s, m]        (S=16 slots)
  logits     = ft_n @ fc_n^T / 0.1             [M, M]
  lse[m]     = logsumexp(logits[m, :])
  semi[m]    = scale * (mean_{n: grp[n]==grp[m]} logits[m, n] - lse[m])
  pos[m]     = scale * (logits[m, m] - lse[m])
  loss       = mean(semi + pos) / 2,   scale = -(0.1/0.07)

Device-side decomposition (all compute on NeuronCores):
  - masked row-sums via a tiny side-GEMM: Q = onehot^T @ fc_n  [16, C],
    P = ft_n @ Q^T  [M, 16], masked_mean[m] = sum_s (onehot/cnt)[m,s]*P[m,s]
  - lse without max-subtraction (|logits| <= 10, fp32-safe)
  - diag via identity-masked fused multiply-reduce on the PSUM logits tile
  - rsqrt via exp(-0.5*ln(ssq)) on ScalarE
  - features cast to bf16 after normalization; GEMMs in bf16 (fp32 PSUM)
Each core handles 4 groups (= 8 consecutive (b,t) frames) and returns
per-partition-row partial sums [128, 1]; the host sums and scales.
"""

import sys
import numpy as np

for p in ("/opt/trn_rl_repo", "/opt/trn_rl_repo/concourse", "/opt/pypackages"):
    if p not in sys.path:
        sys.path.insert(0, p)

GF = 2          # group_frame
S = 16          # slots
N = 1024        # tokens per frame
C = 512         # feature dim
B, T = 8, 8
G = (B * T) // GF            # 32 groups total
M = GF * N                   # 2048 tokens per group
N_CORES = 8
GROUPS_PER_CORE = G // N_CORES   # 4
FRAMES_PER_CORE = GROUPS_PER_CORE * GF  # 8
TEMP = 0.1
BASE_TEMP = 0.07
INV_TEMP = 1.0 / TEMP        # 10.0
SCALE = -(TEMP / BASE_TEMP)

_CACHE = {}


def _build():
    import concourse.mybir as mybir
    from concourse import bacc
    from concourse import masks
    from concourse import bass_isa
    from concourse.tile import TileContext

    dt = mybir.dt
    Alu = mybir.AluOpType
    Act = mybir.ActivationFunctionType

    nc = bacc.Bacc()
    ft_d = nc.declare_dram_parameter("ft", [FRAMES_PER_CORE, N, C], dt.float32, isOutput=False)
    fc_d = nc.declare_dram_parameter("fc", [FRAMES_PER_CORE, N, C], dt.float32, isOutput=False)
    gm_d = nc.declare_dram_parameter("gm", [FRAMES_PER_CORE, S, N], dt.float32, isOutput=False)
    out_d = nc.declare_dram_parameter("out", [128, 2], dt.float32, isOutput=True)

    NT = M // 128       # 16 token tiles per group
    KC = C // 128       # 4 contraction chunks
    NB = M // 512       # 4 psum n-blocks per m-tile

    with TileContext(nc) as tc:
        with (
            tc.tile_pool(name="consts", bufs=1) as consts,
            tc.tile_pool(name="ftT_pool", bufs=2) as ftT_pool,
            tc.tile_pool(name="fcT_pool", bufs=2) as fcT_pool,
            tc.tile_pool(name="qt_pool", bufs=2) as qt_pool,
            tc.tile_pool(name="raw_pool", bufs=18) as raw_pool,
            tc.tile_pool(name="norm_pool", bufs=6) as norm_pool,
            tc.tile_pool(name="stat_pool", bufs=4) as stat_pool,
            tc.tile_pool(name="scr_pool", bufs=2) as scr_pool,
            tc.tile_pool(name="grp_pool", bufs=2) as grp_pool,
            tc.tile_pool(name="col_pool", bufs=3) as col_pool,
            tc.tile_pool(name="acc_pool", bufs=1) as acc_pool,
            tc.tile_pool(name="lg_psum", bufs=3, space="PSUM") as lg_psum,
            tc.tile_pool(name="tp_psum", bufs=2, space="PSUM") as tp_psum,
            tc.tile_pool(name="sm_psum", bufs=2, space="PSUM") as sm_psum,
        ):
            # ---- constants ----
            id_bf16 = consts.tile([128, 128], dt.bfloat16)
            id_f32 = consts.tile([128, 128], dt.float32)
            id16_f32 = consts.tile([S, S], dt.float32)
            id16_bf16 = consts.tile([S, S], dt.bfloat16)
            for t in (id_bf16, id_f32, id16_f32, id16_bf16):
                masks.make_identity(nc, t[:])

            acc = acc_pool.tile([128, 2], dt.float32)
            nc.vector.memset(acc[:], 0.0)

            for g in range(GROUPS_PER_CORE):
                # ============ group-mask phase: onehot + 1/cnt weights ============
                grp_sb = grp_pool.tile([S, M], dt.float32)
                nc.sync.dma_start(
                    out=grp_sb.rearrange("s (f n) -> s f n", f=GF),
                    in_=gm_d[2 * g : 2 * g + 2].rearrange("f s n -> s f n"),
                )
                grpT = grp_pool.tile([128, S * NT], dt.float32)   # token-major [128, 16] x 16
                for j in range(NT):
                    tpg = sm_psum.tile([128, S], dt.float32, tag="sm", name=f"tpg{g}_{j}")
                    nc.tensor.transpose(tpg[:], grp_sb[:, j * 128 : (j + 1) * 128], id16_f32[:])
                    nc.vector.tensor_copy(grpT[:, j * S : (j + 1) * S], tpg[:])
                rowmax = stat_pool.tile([128, NT], dt.float32)
                oh_f32 = grp_pool.tile([128, S * NT], dt.float32)
                oh_bf16 = grp_pool.tile([128, S * NT], dt.bfloat16)
                oh_w = grp_pool.tile([128, S * NT], dt.float32)
                ohsum = stat_pool.tile([128, S], dt.float32)
                cntb = stat_pool.tile([128, S], dt.float32)
                for j in range(NT):
                    sl = slice(j * S, (j + 1) * S)
                    nc.vector.tensor_reduce(
                        out=rowmax[:, j : j + 1], in_=grpT[:, sl],
                        axis=mybir.AxisListType.X, op=Alu.max,
                    )
                    nc.vector.tensor_scalar(
                        out=oh_f32[:, sl], in0=grpT[:, sl],
                        scalar1=rowmax[:, j : j + 1], scalar2=None, op0=Alu.is_equal,
                    )
                    nc.vector.tensor_copy(oh_bf16[:, sl], oh_f32[:, sl])
                    if j == 0:
                        nc.vector.tensor_copy(ohsum[:], oh_f32[:, sl])
                    else:
                        nc.vector.tensor_tensor(out=ohsum[:], in0=ohsum[:], in1=oh_f32[:, sl], op=Alu.add)
                nc.gpsimd.partition_all_reduce(
                    out_ap=cntb[:], in_ap=ohsum[:], channels=128, reduce_op=bass_isa.ReduceOp.add,
                )
                nc.vector.tensor_scalar(out=cntb[:], in0=cntb[:], scalar1=1.0, scalar2=None, op0=Alu.max)
                nc.vector.reciprocal(out=cntb[:], in_=cntb[:])
                for j in range(NT):
                    sl = slice(j * S, (j + 1) * S)
                    nc.vector.tensor_tensor(out=oh_w[:, sl], in0=oh_f32[:, sl], in1=cntb[:], op=Alu.mult)

                # ============ fc phase: normalize + Q-GEMM + transpose ============
                fcT = fcT_pool.tile([128, KC * M], dt.bfloat16)
                qq = sm_psum.tile([S, C], dt.float32, tag="sm", name=f"qq{g}")
                fc_raws = []
                ssq_fc = stat_pool.tile([128, NT], dt.float32)
                for j in range(NT):
                    fc_raw = raw_pool.tile([128, C], dt.float32, tag="raw", name=f"fcraw{g}_{j}")
                    fc_raws.append(fc_raw)
                    nc.sync.dma_start(out=fc_raw[:], in_=fc_d[2 * g + j // 8, (j % 8) * 128 : (j % 8 + 1) * 128, :])
                    sq_scr = scr_pool.tile([128, C], dt.float32, tag="sq")
                    nc.scalar.activation(sq_scr[:], fc_raw[:], Act.Square, accum_out=ssq_fc[:, j : j + 1])
                rn_fc = stat_pool.tile([128, NT], dt.float32)
                nc.vector.tensor_scalar(out=rn_fc[:], in0=ssq_fc[:], scalar1=1e-24, scalar2=None, op0=Alu.max)
                nc.scalar.activation(rn_fc[:], rn_fc[:], Act.Ln)
                nc.scalar.activation(rn_fc[:], rn_fc[:], Act.Exp, scale=-0.5)
                for j in range(NT):
                    fcn = norm_pool.tile([128, C], dt.bfloat16, tag="normed", name=f"fcn{g}_{j}")
                    nc.vector.tensor_scalar(
                        out=fcn[:], in0=fc_raws[j][:], scalar1=rn_fc[:, j : j + 1], scalar2=None, op0=Alu.mult,
                    )
                    nc.tensor.matmul(
                        qq[:], oh_bf16[:, j * S : (j + 1) * S], fcn[:],
                        start=(j == 0), stop=(j == NT - 1),
                    )
                    tp = tp_psum.tile([128, C], dt.float32, tag="tp")
                    for k in range(KC):
                        nc.tensor.matmul(
                            tp[:, k * 128 : (k + 1) * 128], fcn[:, k * 128 : (k + 1) * 128], id_bf16[:],
                            start=True, stop=True,
                        )
                    nc.vector.tensor_copy(
                        fcT.rearrange("p (k m) -> p k m", k=KC)[:, :, j * 128 : (j + 1) * 128],
                        tp.rearrange("p (k m) -> p k m", k=KC),
                    )

                # ============ Q finalize: bf16 + transpose to [C, S] chunks ============
                q_sb = grp_pool.tile([S, C], dt.bfloat16)
                nc.vector.tensor_copy(q_sb[:], qq[:])
                qt = qt_pool.tile([128, KC * S], dt.bfloat16)
                for k in range(KC):
                    tp2 = sm_psum.tile([128, S], dt.float32, tag="sm", name=f"tp2{g}_{k}")
                    nc.tensor.matmul(tp2[:], q_sb[:, k * 128 : (k + 1) * 128], id16_bf16[:], start=True, stop=True)
                    nc.vector.tensor_copy(qt[:, k * S : (k + 1) * S], tp2[:])

                # ============ ft phase: normalize + transpose ============
                ftT = ftT_pool.tile([128, KC * M], dt.bfloat16)
                ft_raws = []
                ssq_ft = stat_pool.tile([128, NT], dt.float32)
                for j in range(NT):
                    ft_raw = raw_pool.tile([128, C], dt.float32, tag="raw", name=f"ftraw{g}_{j}")
                    ft_raws.append(ft_raw)
                    nc.sync.dma_start(out=ft_raw[:], in_=ft_d[2 * g + j // 8, (j % 8) * 128 : (j % 8 + 1) * 128, :])
                    sq_scr = scr_pool.tile([128, C], dt.float32, tag="sq")
                    nc.scalar.activation(sq_scr[:], ft_raw[:], Act.Square, accum_out=ssq_ft[:, j : j + 1])
                rn_ft = stat_pool.tile([128, NT], dt.float32)
                nc.vector.tensor_scalar(out=rn_ft[:], in0=ssq_ft[:], scalar1=1e-24, scalar2=None, op0=Alu.max)
                nc.scalar.activation(rn_ft[:], rn_ft[:], Act.Ln)
                nc.scalar.activation(rn_ft[:], rn_ft[:], Act.Exp, scale=-0.5)
                for j in range(NT):
                    ftn = norm_pool.tile([128, C], dt.bfloat16, tag="normed", name=f"ftn{g}_{j}")
                    nc.vector.tensor_scalar(
                        out=ftn[:], in0=ft_raws[j][:], scalar1=rn_ft[:, j : j + 1], scalar2=None, op0=Alu.mult,
                    )
                    tp = tp_psum.tile([128, C], dt.float32, tag="tp")
                    for k in range(KC):
                        nc.tensor.matmul(
                            tp[:, k * 128 : (k + 1) * 128], ftn[:, k * 128 : (k + 1) * 128], id_bf16[:],
                            start=True, stop=True,
                        )
                    nc.vector.tensor_copy(
                        ftT.rearrange("p (k m) -> p k m", k=KC)[:, :, j * 128 : (j + 1) * 128],
                        tp.rearrange("p (k m) -> p k m", k=KC),
                    )

                # ============ main phase: logits GEMM + LSE + masked means ============
                for i in range(NT):
                    lhs = [ftT[:, k * M + i * 128 : k * M + (i + 1) * 128] for k in range(KC)]
                    lgs = []
                    for nb in range(NB):
                        lg = lg_psum.tile([128, 512], dt.float32, tag="lg", name=f"lg{g}_{i}_{nb}")
                        lgs.append(lg)
                        for k in range(KC):
                            nc.tensor.matmul(
                                lg[:], lhs[k], fcT[:, k * M + nb * 512 : k * M + (nb + 1) * 512],
                                start=(k == 0), stop=(k == KC - 1),
                            )
                    pp = sm_psum.tile([128, S], dt.float32, tag="sm", name=f"pp{g}_{i}")
                    for k in range(KC):
                        nc.tensor.matmul(
                            pp[:], lhs[k], qt[:, k * S : (k + 1) * S],
                            start=(k == 0), stop=(k == KC - 1),
                        )
                    # diagonal (cosine units) from the block that contains it
                    diagc = col_pool.tile([128, 1], dt.float32, tag="diagc")
                    ttr_scr = scr_pool.tile([128, 128], dt.float32, tag="ttr")
                    nc.vector.tensor_tensor(
                        out=ttr_scr[:], in0=lgs[i // 4][:, (i % 4) * 128 : (i % 4 + 1) * 128],
                        in1=id_f32[:], op=Alu.mult,
                    )
                    nc.vector.tensor_reduce(
                        out=diagc[:], in_=ttr_scr[:], axis=mybir.AxisListType.X, op=Alu.add,
                    )
                    # exp (scale=1/T) + row-sum accumulation
                    scols = col_pool.tile([128, NB], dt.float32, tag="scols")
                    for nb in range(NB):
                        exp_scr = scr_pool.tile([128, 512], dt.bfloat16, tag="exp")
                        nc.scalar.activation(
                            exp_scr[:], lgs[nb][:], Act.Exp, scale=INV_TEMP,
                            accum_out=scols[:, nb : nb + 1],
                        )
                    stot = col_pool.tile([128, 1], dt.float32, tag="stot")
                    nc.vector.tensor_reduce(out=stot[:], in_=scols[:], axis=mybir.AxisListType.X, op=Alu.add)
                    lse = col_pool.tile([128, 1], dt.float32, tag="lse")
                    nc.scalar.activation(lse[:], stot[:], Act.Ln)
                    # masked mean (cosine units): sum_s oh_w * P
                    mavg = col_pool.tile([128, 1], dt.float32, tag="mavg")
                    pttr_scr = scr_pool.tile([128, S], dt.float32, tag="pttr")
                    nc.vector.tensor_tensor(
                        out=pttr_scr[:], in0=pp[:], in1=oh_w[:, i * S : (i + 1) * S], op=Alu.mult,
                    )
                    nc.vector.tensor_reduce(
                        out=mavg[:], in_=pttr_scr[:], axis=mybir.AxisListType.X, op=Alu.add,
                    )
                    # acc col0 += mavg + diag (cosine units); acc col1 += lse
                    t1 = col_pool.tile([128, 1], dt.float32, tag="t1")
                    nc.vector.tensor_tensor(out=t1[:], in0=mavg[:], in1=diagc[:], op=Alu.add)
                    nc.vector.tensor_tensor(out=acc[:, 0:1], in0=acc[:, 0:1], in1=t1[:], op=Alu.add)
                    nc.vector.tensor_tensor(out=acc[:, 1:2], in0=acc[:, 1:2], in1=lse[:], op=Alu.add)

            nc.sync.dma_start(out=out_d[:, :], in_=acc[:])

    nc.compile()
    return nc


def kernel(feat_trainable: np.ndarray, feat_criterion: np.ndarray, grp_masks: np.ndarray) -> np.ndarray:
    from concourse.bass_utils import run_bass_kernel_spmd

    if "nc" not in _CACHE:
        _CACHE["nc"] = _build()
    nc = _CACHE["nc"]

    ft = np.ascontiguousarray(np.asarray(feat_trainable, dtype=np.float32).reshape(B * T, N, C))
    fc = np.ascontiguousarray(np.asarray(feat_criterion, dtype=np.float32).reshape(B * T, N, C))
    gm = np.ascontiguousarray(np.asarray(grp_masks, dtype=np.float32).reshape(B * T, S, N))

    in_maps = []
    for c in range(N_CORES):
        fr = slice(c * FRAMES_PER_CORE, (c + 1) * FRAMES_PER_CORE)
        in_maps.append({
            "ft": np.ascontiguousarray(ft[fr]),
            "fc": np.ascontiguousarray(fc[fr]),
            "gm": np.ascontiguousarray(gm[fr]),
        })

    import time
    last_err = None
    for attempt in range(4):
        try:
            res = run_bass_kernel_spmd(nc, in_maps, list(range(N_CORES)))
            break
        except Exception as e:  # wedged-device recovery: wait and retry
            last_err = e
            time.sleep(20 + 25 * attempt)
    else:
        raise last_err
    total = np.float64(0.0)
    for c in range(N_CORES):
        o = np.asarray(res.results[c]["out"], dtype=np.float64)
        total += INV_TEMP * o[:, 0].sum() - 2.0 * o[:, 1].sum()
    loss = SCALE * total / (G * M) / 2.0
    return np.asarray(loss, dtype=np.float32)


if __name__ == "__main__":
    # build-only smoke test
    nc = _build()
    print("build OK")
